# revision 4
# baseline (speedup 1.0000x reference)
"""Trainium2 Bass kernel for the Jacobian-log-det-squared loss.

Reference computation (per voxel of a (B,C=3,D,H,W) displacement field):
    J[j,i] = d(u_i)/d(x_j) + delta_ij   (numpy-style gradient: central in the
             interior, one-sided at the boundary), out = log(det(J))**2.

Strategy
--------
* Shard over (B=2) x (H quarters=4) -> 8 cores, pure data parallel. Each core
  receives a halo-padded slab (3, 128, 50, 194) and computes (128, 48, 192).
* Host pads ghost cells (2*x0 - x1) along H and W so one-sided boundary
  differences become central differences of the padded array - the device
  kernel is a uniform interior stencil with no edge special-casing.
* D axis (=128) sits on SBUF partitions; the D-gradient (including its
  boundary rows) is a banded 128x128 fp32 matmul on the TensorEngine over
  the whole input chunk (4 matmuls of <=512 free elems into one PSUM tile).
* H/W gradients: the fp32 diagonal ones (G11, G22) are shifted-AP subtracts
  on DVE; the four off-diagonal ones are fp16 subtracts at DVE 2x mode over
  GPSIMD-cast fp16 copies of x (cast layouts chosen so every fp16 operand is
  4-byte aligned).
* det(I+G) - 1 = c1 + c2 + c3: dominant trace term c1 in fp32, small
  quadratic/cubic terms in fp16. Pure adds are offloaded to the DMA engines
  (SWDGE accum_op=add); scale+add pairs are fused scalar_tensor_tensor ops.
  All gradients are carried as 2*G and the scale is folded into the final
  ScalarEngine activation: log(0.5 * zf + 1), then Square.
"""

import sys

sys.path.insert(0, "/opt/trn_rl_repo")

import numpy as np
from contextlib import ExitStack

import concourse.bass as bass  # noqa: F401
import concourse.tile as tile
from concourse import bacc, mybir
from concourse.bass_utils import run_bass_kernel_spmd
from concourse.bass_interp import get_hw_module

f32 = mybir.dt.float32
f16 = mybir.dt.float16
Act = mybir.ActivationFunctionType
Alu = mybir.AluOpType

B, C, D, H, W = 2, 3, 128, 192, 192
NCORES = 8
HQ = 4                 # H quarters (cores = B * HQ)
HL = H // HQ           # 48 output H rows per core
HCH = 8                # output H rows per chunk
NCH = HL // HCH        # chunks per core
WP = W + 2             # W padded with ghost columns
HIN = HCH + 2          # input rows per chunk (halo)
FI = HIN * WP          # flat input chunk size (1940)
FD = HCH * W           # flat output chunk size (1536)

_CACHE = {}


def _dmat2() -> np.ndarray:
    """Banded difference matrix (scaled by 2): out[m] = sum_k dmat[k,m] x[k]
    equals 2 * (numpy-gradient of x along D at m)."""
    m = np.zeros((D, D), np.float32)
    for j in range(1, D - 1):
        m[j - 1, j] = -1.0
        m[j + 1, j] = 1.0
    m[0, 0], m[1, 0] = -2.0, 2.0
    m[D - 2, D - 1], m[D - 1, D - 1] = -2.0, 2.0
    return m


def _build_program():
    nc = bacc.Bacc("TRN2", target_bir_lowering=False, debug=False,
                   num_devices=NCORES)
    x_in = nc.dram_tensor("x", [C, D, HL + 2, WP], f32,
                          kind="ExternalInput").ap()
    dm_in = nc.dram_tensor("dmat", [D, D], f32, kind="ExternalInput").ap()
    out_t = nc.dram_tensor("out", [D, HL, W], f32, kind="ExternalOutput").ap()

    with tile.TileContext(nc) as tc, ExitStack() as ctx:
        const_pool = ctx.enter_context(tc.tile_pool(name="const", bufs=1))
        xin_pool = ctx.enter_context(tc.tile_pool(name="xin", bufs=2))
        ps_pool = ctx.enter_context(
            tc.tile_pool(name="ps", bufs=2, space="PSUM"))
        r32_pool = ctx.enter_context(tc.tile_pool(name="r32", bufs=2))
        h16_pool = ctx.enter_context(tc.tile_pool(name="h16", bufs=2))
        x16_pool = ctx.enter_context(tc.tile_pool(name="x16", bufs=2))
        tmp_pool = ctx.enter_context(tc.tile_pool(name="t16", bufs=8))

        dmat = const_pool.tile([D, D], f32)
        nc.sync.dma_start(dmat[:], dm_in[:])

        for chi in range(NCH):
            h0 = chi * HCH  # top input row of this chunk (padded coords)

            xt, xv = [], []
            for c in range(C):
                t = xin_pool.tile([D, FI], f32, tag=f"x{c}", name=f"xin{c}")
                v = t[:].rearrange("p (h w) -> p h w", h=HIN, w=WP)
                nc.sync.dma_start(v, x_in[c, :, h0:h0 + HIN, :])
                xt.append(t)
                xv.append(v)

            def t32(tag):
                return r32_pool.tile([D, FD], f32, tag=tag, name="r32_" + tag)

            def t16(tag):
                return h16_pool.tile([D, FD], f16, tag=tag, name="h16_" + tag)

            def tmp():
                return tmp_pool.tile([D, FD], f16, tag="tmp", name="tmp16")

            def v3(t, h=HCH, w=W):
                return t[:].rearrange("p (h w) -> p h w", h=h, w=w)

            # ---- fp16 copies of x for the off-diagonal diffs ------------
            # xs: all rows, w interior (for H-diffs; offsets stay even)
            # xf: interior rows, all w (for W-diffs; reads at w and w+2)
            xs0 = x16_pool.tile([D, HIN * W], f16, tag="xs0", name="xs0")
            nc.gpsimd.tensor_copy(v3(xs0, h=HIN), xv[0][:, :, 1:WP - 1])
            xs2 = x16_pool.tile([D, HIN * W], f16, tag="xs2", name="xs2")
            nc.gpsimd.tensor_copy(v3(xs2, h=HIN), xv[2][:, :, 1:WP - 1])
            xf0 = x16_pool.tile([D, HCH * WP], f16, tag="xf0", name="xf0")
            nc.gpsimd.tensor_copy(v3(xf0, w=WP), xv[0][:, 1:HIN - 1, :])
            xf1 = x16_pool.tile([D, HCH * WP], f16, tag="xf1", name="xf1")
            nc.gpsimd.tensor_copy(v3(xf1, w=WP), xv[1][:, 1:HIN - 1, :])

            # ---- raw differences (values are 2*G) -----------------------
            e_r = t32("e")   # 2*G11 = H-diff of u1 (fp32)
            nc.vector.tensor_sub(v3(e_r), xv[1][:, 2:HIN, 1:WP - 1],
                                 xv[1][:, 0:HIN - 2, 1:WP - 1])
            i_r = t32("i")   # 2*G22 = W-diff of u2 (fp32)
            nc.vector.tensor_sub(v3(i_r), xv[2][:, 1:HIN - 1, 2:WP],
                                 xv[2][:, 1:HIN - 1, 0:WP - 2])
            d16 = t16("d")   # 2*G10 (fp16, 2x mode)
            vs0 = v3(xs0, h=HIN)
            nc.vector.tensor_sub(v3(d16), vs0[:, 2:HIN, :],
                                 vs0[:, 0:HIN - 2, :])
            fd16 = t16("f")  # 2*G12
            vs2 = v3(xs2, h=HIN)
            nc.vector.tensor_sub(v3(fd16), vs2[:, 2:HIN, :],
                                 vs2[:, 0:HIN - 2, :])
            g16 = t16("g")   # 2*G20
            vf0 = v3(xf0, w=WP)
            nc.vector.tensor_sub(v3(g16), vf0[:, :, 2:WP], vf0[:, :, 0:WP - 2])
            hd16 = t16("h")  # 2*G21
            vf1 = v3(xf1, w=WP)
            nc.vector.tensor_sub(v3(hd16), vf1[:, :, 2:WP],
                                 vf1[:, :, 0:WP - 2])

            # fp16 copies of the diagonal diffs (ScalarE)
            e16 = t16("e16")
            nc.scalar.copy(e16[:], e_r[:])
            i16 = t16("i16")
            nc.scalar.copy(i16[:], i_r[:])

            # T = 2*(G11+G22), accumulated into e_r's tile by the DMA CCE
            nc.gpsimd.dma_start(e_r[:], i_r[:], accum_op=Alu.add)
            T2 = e_r
            T16 = t16("T16")
            nc.scalar.copy(T16[:], T2[:])

            # ---- TensorEngine: D-gradient rows --------------------------
            # a,b,c = 2*G00, 2*G01, 2*G02 over the full input chunk
            zc2 = t32("z")
            ab16 = [t16(t) for t in ("a16", "b16", "c16")]
            for c in range(C):
                ps = ps_pool.tile([D, FI], f32, tag="ps", name="ps")
                for k in range(0, FI, 512):
                    n = min(512, FI - k)
                    nc.tensor.matmul(ps[:, k:k + n], dmat[:],
                                     xt[c][:, k:k + n])
                ps_int = v3(ps, h=HIN, w=WP)[:, 1:HIN - 1, 1:WP - 1]
                nc.scalar.copy(v3(ab16[c]), ps_int)
                if c == 0:
                    nc.vector.tensor_add(v3(zc2), ps_int, v3(T2))
            a16, b16, c16 = ab16

            # ---- fp16 product chain (DVE 2x) ----------------------------
            def mul(o, x, y):
                nc.vector.tensor_mul(o[:], x[:], y[:])

            def sub(o, x, y):
                nc.vector.tensor_sub(o[:], x[:], y[:])

            def acc(o, x):  # o += x on the DMA engines
                nc.gpsimd.dma_start(o[:], x[:], accum_op=Alu.add)

            pa, pb = tmp(), tmp()
            mul(pa, e16, i16)
            mul(pb, fd16, hd16)
            N0 = tmp()
            sub(N0, pa, pb)            # 4*(ei - fh)

            pa, pb = tmp(), tmp()
            mul(pa, d16, i16)
            mul(pb, fd16, g16)
            N1 = tmp()
            sub(N1, pa, pb)            # 4*(di - fg)
            Q2 = tmp()
            mul(Q2, b16, N1)           # 8*b(di - fg)

            pa, pb = tmp(), tmp()
            mul(pa, d16, hd16)
            mul(pb, e16, g16)
            N2 = tmp()
            sub(N2, pa, pb)            # 4*(dh - eg)
            Q3 = tmp()
            mul(Q3, c16, N2)           # 8*c(dh - eg)

            Q1 = tmp()
            mul(Q1, a16, N0)           # 8*a(ei - fh)
            Q4 = tmp()
            sub(Q4, Q1, Q2)
            acc(Q4, Q3)                # R3 = 8*c3 (in Q4's tile)
            R3 = Q4

            U1 = tmp()
            mul(U1, a16, T16)          # 4*a(e+i)
            U2 = tmp()
            mul(U2, b16, d16)          # 4*bd
            U3 = tmp()
            mul(U3, c16, g16)          # 4*cg
            acc(U2, U3)                # U4 = 4*(bd+cg) (in U2's tile)
            U5 = tmp()
            sub(U5, U1, U2)            # 4*c2 - 4*(ei-fh)
            acc(U5, N0)                # V1 = 4*c2 (in U5's tile)

            V2 = tmp()                 # 4*(c2+c3) = V1 + 0.5*R3
            nc.vector.scalar_tensor_tensor(V2[:], R3[:], 0.5, U5[:],
                                           Alu.mult, Alu.add)

            # zf = 2*(det-1) = zc2 + 0.5*V2 ; then log(0.5*zf+1), square
            zf2 = t32("i")  # i_r is dead by now; reuse its slots
            nc.vector.scalar_tensor_tensor(zf2[:], V2[:], 0.5, zc2[:],
                                           Alu.mult, Alu.add)
            nc.scalar.activation(zf2[:], zf2[:], Act.Ln, bias=1.0, scale=0.5)
            nc.scalar.activation(zf2[:], zf2[:], Act.Square)

            nc.sync.dma_start(
                out_t[:, chi * HCH:(chi + 1) * HCH, :], v3(zf2))

    nc.compile()
    nc.m = get_hw_module(nc.m)
    return nc


def _get_program():
    if "nc" not in _CACHE:
        _CACHE["nc"] = _build_program()
    return _CACHE["nc"]


def make_in_maps(x: np.ndarray):
    x = np.asarray(x, dtype=np.float32)
    # ghost cells: 2*x[edge] - x[edge+1] makes the central difference of the
    # padded array equal the one-sided boundary difference of the original
    xw = np.concatenate(
        [2.0 * x[..., :1] - x[..., 1:2], x,
         2.0 * x[..., -1:] - x[..., -2:-1]], axis=-1)
    xh = np.concatenate(
        [2.0 * xw[:, :, :, :1] - xw[:, :, :, 1:2], xw,
         2.0 * xw[:, :, :, -1:] - xw[:, :, :, -2:-1]], axis=3)
    dmat = _dmat2()
    in_maps = []
    for core in range(NCORES):
        b, hq = divmod(core, HQ)
        slab = np.ascontiguousarray(xh[b, :, :, hq * HL: hq * HL + HL + 2, :])
        in_maps.append({"x": slab, "dmat": dmat})
    return in_maps


def kernel(x: np.ndarray) -> np.ndarray:
    x = np.asarray(x, dtype=np.float32)
    assert x.shape == (B, C, D, H, W)
    in_maps = make_in_maps(x)
    nc = _get_program()
    res = run_bass_kernel_spmd(nc, in_maps, list(range(NCORES)))
    out = np.empty((B, D, H, W), np.float32)
    for core in range(NCORES):
        b, hq = divmod(core, HQ)
        out[b, :, hq * HL:(hq + 1) * HL, :] = res.results[core]["out"]
    return out


if __name__ == "__main__":
    rng = np.random.default_rng(0)
    xt = (rng.standard_normal((B, C, D, H, W)) * 0.05).astype(np.float32)
    y = kernel(xt)
    print("out", y.shape, y.dtype, float(y.mean()))


# revision 5
# speedup vs baseline: 1.3002x; 1.3002x over previous
"""Trainium2 Bass kernel for the Jacobian-log-det-squared loss.

Reference computation (per voxel of a (B,C=3,D,H,W) displacement field):
    J[j,i] = d(u_i)/d(x_j) + delta_ij   (numpy-style gradient: central in the
             interior, one-sided at the boundary), out = log(det(J))**2.

Strategy
--------
* Shard over (B=2) x (H quarters=4) -> 8 cores, pure data parallel. Each core
  receives a halo-padded slab (3, 128, 50, 194) and computes (128, 48, 192).
* Host pads ghost cells (2*x0 - x1) along H and W so one-sided boundary
  differences become central differences of the padded array - the device
  kernel is a uniform interior stencil with no edge special-casing.
* D axis (=128) sits on SBUF partitions; the D-gradient (including its
  boundary rows) is a banded 128x128 fp32 matmul on the TensorEngine over
  the whole input chunk (4 matmuls of <=512 free elems into one PSUM tile).
* H/W gradients: the fp32 diagonal ones (G11, G22) are shifted-AP subtracts
  on DVE; the four off-diagonal ones are fp16 subtracts at DVE 2x mode over
  GPSIMD-cast fp16 copies of x (cast layouts chosen so every fp16 operand is
  4-byte aligned).
* det(I+G) - 1 = c1 + c2 + c3: dominant trace term c1 in fp32, small
  quadratic/cubic terms in fp16. Pure adds are offloaded to the DMA engines
  (SWDGE accum_op=add); scale+add pairs are fused scalar_tensor_tensor ops.
  All gradients are carried as 2*G and the scale is folded into the final
  ScalarEngine activation: log(0.5 * zf + 1), then Square.
"""

import sys

sys.path.insert(0, "/opt/trn_rl_repo")

import numpy as np
from contextlib import ExitStack

import concourse.bass as bass  # noqa: F401
import concourse.tile as tile
from concourse import bacc, mybir
from concourse.bass_utils import run_bass_kernel_spmd
from concourse.bass_interp import get_hw_module

f32 = mybir.dt.float32
f16 = mybir.dt.float16
Act = mybir.ActivationFunctionType
Alu = mybir.AluOpType

B, C, D, H, W = 2, 3, 128, 192, 192
NCORES = 8
HQ = 4                 # H quarters (cores = B * HQ)
HL = H // HQ           # 48 output H rows per core
HCH = 8                # output H rows per chunk
NCH = HL // HCH        # chunks per core
WP = W + 2             # W padded with ghost columns
HIN = HCH + 2          # input rows per chunk (halo)
FI = HIN * WP          # flat input chunk size (1940)
FD = HCH * W           # flat output chunk size (1536)

_CACHE = {}


def _dmat2() -> np.ndarray:
    """Banded difference matrix (scaled by 2): out[m] = sum_k dmat[k,m] x[k]
    equals 2 * (numpy-gradient of x along D at m)."""
    m = np.zeros((D, D), np.float32)
    for j in range(1, D - 1):
        m[j - 1, j] = -1.0
        m[j + 1, j] = 1.0
    m[0, 0], m[1, 0] = -2.0, 2.0
    m[D - 2, D - 1], m[D - 1, D - 1] = -2.0, 2.0
    return m


def _build_program():
    nc = bacc.Bacc("TRN2", target_bir_lowering=False, debug=False,
                   num_devices=NCORES)
    x_in = nc.dram_tensor("x", [C, D, HL + 2, WP], f32,
                          kind="ExternalInput").ap()
    dm_in = nc.dram_tensor("dmat", [D, D], f32, kind="ExternalInput").ap()
    out_t = nc.dram_tensor("out", [D, HL, W], f32, kind="ExternalOutput").ap()

    with tile.TileContext(nc) as tc, ExitStack() as ctx:
        const_pool = ctx.enter_context(tc.tile_pool(name="const", bufs=1))
        xin_pool = ctx.enter_context(tc.tile_pool(name="xin", bufs=2))
        ps_pool = ctx.enter_context(
            tc.tile_pool(name="ps", bufs=2, space="PSUM"))
        r32_pool = ctx.enter_context(tc.tile_pool(name="r32", bufs=2))
        h16_pool = ctx.enter_context(tc.tile_pool(name="h16", bufs=2))
        tmp_pool = ctx.enter_context(tc.tile_pool(name="t16", bufs=8))

        dmat = const_pool.tile([D, D], f32)
        nc.sync.dma_start(dmat[:], dm_in[:])

        for chi in range(NCH):
            h0 = chi * HCH  # top input row of this chunk (padded coords)

            xt, xv = [], []
            for c in range(C):
                t = xin_pool.tile([D, FI], f32, tag=f"x{c}", name=f"xin{c}")
                v = t[:].rearrange("p (h w) -> p h w", h=HIN, w=WP)
                nc.sync.dma_start(v, x_in[c, :, h0:h0 + HIN, :])
                xt.append(t)
                xv.append(v)

            def t32(tag):
                return r32_pool.tile([D, FD], f32, tag=tag, name="r32_" + tag)

            def t16(tag):
                return h16_pool.tile([D, FD], f16, tag=tag, name="h16_" + tag)

            def tmp():
                return tmp_pool.tile([D, FD], f16, tag="tmp", name="tmp16")

            def v3(t, h=HCH, w=W):
                return t[:].rearrange("p (h w) -> p h w", h=h, w=w)

            # ---- raw differences (values are 2*G) -----------------------
            def hdiff(c):
                return (xv[c][:, 2:HIN, 1:WP - 1],
                        xv[c][:, 0:HIN - 2, 1:WP - 1])

            def wdiff(c):
                return (xv[c][:, 1:HIN - 1, 2:WP],
                        xv[c][:, 1:HIN - 1, 0:WP - 2])

            e_r = t32("e")   # 2*G11 = H-diff of u1 (fp32)
            nc.vector.tensor_sub(v3(e_r), *hdiff(1))
            i_r = t32("i")   # 2*G22 = W-diff of u2 (fp32)
            nc.vector.tensor_sub(v3(i_r), *wdiff(2))
            d16 = t16("d")   # 2*G10 (fp16 out)
            nc.vector.tensor_sub(v3(d16), *hdiff(0))
            fd16 = t16("f")  # 2*G12
            nc.vector.tensor_sub(v3(fd16), *hdiff(2))
            g16 = t16("g")   # 2*G20
            nc.vector.tensor_sub(v3(g16), *wdiff(0))
            hd16 = t16("h")  # 2*G21
            nc.vector.tensor_sub(v3(hd16), *wdiff(1))

            # fp16 copies of the diagonal diffs (ScalarE)
            e16 = t16("e16")
            nc.scalar.copy(e16[:], e_r[:])
            i16 = t16("i16")
            nc.scalar.copy(i16[:], i_r[:])

            # T = 2*(G11+G22), accumulated into e_r's tile by the DMA CCE
            nc.gpsimd.dma_start(e_r[:], i_r[:], accum_op=Alu.add)
            T2 = e_r
            T16 = t16("T16")
            nc.scalar.copy(T16[:], T2[:])

            # ---- TensorEngine: D-gradient rows --------------------------
            # a,b,c = 2*G00, 2*G01, 2*G02 over the full input chunk
            zc2 = t32("z")
            ab16 = [t16(t) for t in ("a16", "b16", "c16")]
            for c in range(C):
                ps = ps_pool.tile([D, FI], f32, tag="ps", name="ps")
                for k in range(0, FI, 512):
                    n = min(512, FI - k)
                    nc.tensor.matmul(ps[:, k:k + n], dmat[:],
                                     xt[c][:, k:k + n])
                ps_int = v3(ps, h=HIN, w=WP)[:, 1:HIN - 1, 1:WP - 1]
                nc.scalar.copy(v3(ab16[c]), ps_int)
                if c == 0:
                    nc.vector.tensor_add(v3(zc2), ps_int, v3(T2))
            a16, b16, c16 = ab16

            # ---- fp16 product chain (DVE 2x) ----------------------------
            def mul(o, x, y):
                nc.vector.tensor_mul(o[:], x[:], y[:])

            def sub(o, x, y):
                nc.vector.tensor_sub(o[:], x[:], y[:])

            def acc(o, x):  # o += x on the DMA engines
                nc.gpsimd.dma_start(o[:], x[:], accum_op=Alu.add)

            pa, pb = tmp(), tmp()
            mul(pa, e16, i16)
            mul(pb, fd16, hd16)
            N0 = tmp()
            sub(N0, pa, pb)            # 4*(ei - fh)

            pa, pb = tmp(), tmp()
            mul(pa, d16, i16)
            mul(pb, fd16, g16)
            N1 = tmp()
            sub(N1, pa, pb)            # 4*(di - fg)
            Q2 = tmp()
            mul(Q2, b16, N1)           # 8*b(di - fg)

            pa, pb = tmp(), tmp()
            mul(pa, d16, hd16)
            mul(pb, e16, g16)
            N2 = tmp()
            sub(N2, pa, pb)            # 4*(dh - eg)
            Q3 = tmp()
            mul(Q3, c16, N2)           # 8*c(dh - eg)

            Q1 = tmp()
            mul(Q1, a16, N0)           # 8*a(ei - fh)
            Q4 = tmp()
            sub(Q4, Q1, Q2)
            acc(Q4, Q3)                # R3 = 8*c3 (in Q4's tile)
            R3 = Q4

            U1 = tmp()
            mul(U1, a16, T16)          # 4*a(e+i)
            U2 = tmp()
            mul(U2, b16, d16)          # 4*bd
            U3 = tmp()
            mul(U3, c16, g16)          # 4*cg
            acc(U2, U3)                # U4 = 4*(bd+cg) (in U2's tile)
            U5 = tmp()
            sub(U5, U1, U2)            # 4*c2 - 4*(ei-fh)
            acc(U5, N0)                # V1 = 4*c2 (in U5's tile)

            V2 = tmp()                 # 4*(c2+c3) = V1 + 0.5*R3
            nc.vector.scalar_tensor_tensor(V2[:], R3[:], 0.5, U5[:],
                                           Alu.mult, Alu.add)

            # zf = 2*(det-1) = zc2 + 0.5*V2 ; then log(0.5*zf+1), square
            zf2 = t32("i")  # i_r is dead by now; reuse its slots
            nc.vector.scalar_tensor_tensor(zf2[:], V2[:], 0.5, zc2[:],
                                           Alu.mult, Alu.add)
            nc.scalar.activation(zf2[:], zf2[:], Act.Ln, bias=1.0, scale=0.5)
            nc.scalar.activation(zf2[:], zf2[:], Act.Square)

            nc.sync.dma_start(
                out_t[:, chi * HCH:(chi + 1) * HCH, :], v3(zf2))

    nc.compile()
    nc.m = get_hw_module(nc.m)
    return nc


def _get_program():
    if "nc" not in _CACHE:
        _CACHE["nc"] = _build_program()
    return _CACHE["nc"]


def make_in_maps(x: np.ndarray):
    x = np.asarray(x, dtype=np.float32)
    # ghost cells: 2*x[edge] - x[edge+1] makes the central difference of the
    # padded array equal the one-sided boundary difference of the original
    xw = np.concatenate(
        [2.0 * x[..., :1] - x[..., 1:2], x,
         2.0 * x[..., -1:] - x[..., -2:-1]], axis=-1)
    xh = np.concatenate(
        [2.0 * xw[:, :, :, :1] - xw[:, :, :, 1:2], xw,
         2.0 * xw[:, :, :, -1:] - xw[:, :, :, -2:-1]], axis=3)
    dmat = _dmat2()
    in_maps = []
    for core in range(NCORES):
        b, hq = divmod(core, HQ)
        slab = np.ascontiguousarray(xh[b, :, :, hq * HL: hq * HL + HL + 2, :])
        in_maps.append({"x": slab, "dmat": dmat})
    return in_maps


def kernel(x: np.ndarray) -> np.ndarray:
    x = np.asarray(x, dtype=np.float32)
    assert x.shape == (B, C, D, H, W)
    in_maps = make_in_maps(x)
    nc = _get_program()
    res = run_bass_kernel_spmd(nc, in_maps, list(range(NCORES)))
    out = np.empty((B, D, H, W), np.float32)
    for core in range(NCORES):
        b, hq = divmod(core, HQ)
        out[b, :, hq * HL:(hq + 1) * HL, :] = res.results[core]["out"]
    return out


if __name__ == "__main__":
    rng = np.random.default_rng(0)
    xt = (rng.standard_normal((B, C, D, H, W)) * 0.05).astype(np.float32)
    y = kernel(xt)
    print("out", y.shape, y.dtype, float(y.mean()))


# revision 6
# speedup vs baseline: 1.7186x; 1.3218x over previous
"""Trainium2 Bass kernel for the Jacobian-log-det-squared loss.

Reference computation (per voxel of a (B,C=3,D,H,W) displacement field):
    J[j,i] = d(u_i)/d(x_j) + delta_ij   (numpy-style gradient: central in the
             interior, one-sided at the boundary), out = log(det(J))**2.

Strategy
--------
* Shard over (B=2) x (H quarters=4) -> 8 cores, pure data parallel. Each core
  receives a halo-padded slab (3, 128, 50, 194) and computes (128, 48, 192).
* Host pads ghost cells (2*x0 - x1) along H and W so one-sided boundary
  differences become central differences of the padded array - the device
  kernel is a uniform interior stencil with no edge special-casing.
* D axis (=128) sits on SBUF partitions; the D-gradient (including its
  boundary rows) is a banded 128x128 fp32 matmul on the TensorEngine over
  the whole input chunk (4 matmuls of <=512 free elems into one PSUM tile).
* H/W gradients: the fp32 diagonal ones (G11, G22) are shifted-AP subtracts
  on DVE; the four off-diagonal ones are fp16 subtracts at DVE 2x mode over
  GPSIMD-cast fp16 copies of x (cast layouts chosen so every fp16 operand is
  4-byte aligned).
* det(I+G) - 1 = c1 + c2 + c3: dominant trace term c1 in fp32, small
  quadratic/cubic terms in fp16. Pure adds are offloaded to the DMA engines
  (SWDGE accum_op=add); scale+add pairs are fused scalar_tensor_tensor ops.
  All gradients are carried as 2*G and the scale is folded into the final
  ScalarEngine activation: log(0.5 * zf + 1), then Square.
"""

import sys

sys.path.insert(0, "/opt/trn_rl_repo")

import numpy as np
from contextlib import ExitStack

import concourse.bass as bass  # noqa: F401
import concourse.tile as tile
from concourse import bacc, mybir
from concourse.bass_utils import run_bass_kernel_spmd
from concourse.bass_interp import get_hw_module

f32 = mybir.dt.float32
f16 = mybir.dt.float16
Act = mybir.ActivationFunctionType
Alu = mybir.AluOpType

B, C, D, H, W = 2, 3, 128, 192, 192
NCORES = 8
HQ = 4                 # H quarters (cores = B * HQ)
HL = H // HQ           # 48 output H rows per core
HCH = 8                # output H rows per chunk
NCH = HL // HCH        # chunks per core
WP = W + 2             # W padded with ghost columns
HIN = HCH + 2          # input rows per chunk (halo)
FI = HIN * WP          # flat input chunk size (1940)
FD = HCH * W           # flat output chunk size (1536)

_CACHE = {}


def _dmat2() -> np.ndarray:
    """Banded difference matrix (scaled by 2): out[m] = sum_k dmat[k,m] x[k]
    equals 2 * (numpy-gradient of x along D at m)."""
    m = np.zeros((D, D), np.float32)
    for j in range(1, D - 1):
        m[j - 1, j] = -1.0
        m[j + 1, j] = 1.0
    m[0, 0], m[1, 0] = -2.0, 2.0
    m[D - 2, D - 1], m[D - 1, D - 1] = -2.0, 2.0
    return m


def _build_program():
    nc = bacc.Bacc("TRN2", target_bir_lowering=False, debug=False,
                   num_devices=NCORES)
    x_in = nc.dram_tensor("x", [C, D, HL + 2, WP], f32,
                          kind="ExternalInput").ap()
    dm_in = nc.dram_tensor("dmat", [D, D], f32, kind="ExternalInput").ap()
    out_t = nc.dram_tensor("out", [D, HL, W], f32, kind="ExternalOutput").ap()

    with tile.TileContext(nc) as tc, ExitStack() as ctx:
        const_pool = ctx.enter_context(tc.tile_pool(name="const", bufs=1))
        xin_pool = ctx.enter_context(tc.tile_pool(name="xin", bufs=2))
        ps_pool = ctx.enter_context(
            tc.tile_pool(name="ps", bufs=2, space="PSUM"))
        r32_pool = ctx.enter_context(tc.tile_pool(name="r32", bufs=2))
        h16_pool = ctx.enter_context(tc.tile_pool(name="h16", bufs=2))
        tmp_pool = ctx.enter_context(tc.tile_pool(name="t16", bufs=8))

        dmat = const_pool.tile([D, D], f32)
        nc.sync.dma_start(dmat[:], dm_in[:])

        for chi in range(NCH):
            h0 = chi * HCH  # top input row of this chunk (padded coords)

            xt, xv = [], []
            for c in range(C):
                t = xin_pool.tile([D, FI], f32, tag=f"x{c}", name=f"xin{c}")
                v = t[:].rearrange("p (h w) -> p h w", h=HIN, w=WP)
                nc.sync.dma_start(v, x_in[c, :, h0:h0 + HIN, :])
                xt.append(t)
                xv.append(v)

            def t32(tag):
                return r32_pool.tile([D, FD], f32, tag=tag, name="r32_" + tag)

            def t16(tag):
                return h16_pool.tile([D, FD], f16, tag=tag, name="h16_" + tag)

            def tmp():
                return tmp_pool.tile([D, FD], f16, tag="tmp", name="tmp16")

            def v3(t, h=HCH, w=W):
                return t[:].rearrange("p (h w) -> p h w", h=h, w=w)

            # ---- raw differences (values are 2*G) -----------------------
            def hdiff(c):
                return (xv[c][:, 2:HIN, 1:WP - 1],
                        xv[c][:, 0:HIN - 2, 1:WP - 1])

            def wdiff(c):
                return (xv[c][:, 1:HIN - 1, 2:WP],
                        xv[c][:, 1:HIN - 1, 0:WP - 2])

            e_r = t32("e")   # 2*G11 = H-diff of u1 (fp32)
            nc.vector.tensor_sub(v3(e_r), *hdiff(1))
            i_r = t32("i")   # 2*G22 = W-diff of u2 (fp32)
            nc.vector.tensor_sub(v3(i_r), *wdiff(2))
            d16 = t16("d")   # 2*G10 (fp16 out)
            nc.vector.tensor_sub(v3(d16), *hdiff(0))
            fd16 = t16("f")  # 2*G12
            nc.vector.tensor_sub(v3(fd16), *hdiff(2))
            g16 = t16("g")   # 2*G20
            nc.vector.tensor_sub(v3(g16), *wdiff(0))
            hd16 = t16("h")  # 2*G21
            nc.vector.tensor_sub(v3(hd16), *wdiff(1))

            # fp16 copies of the diagonal diffs (ScalarE)
            e16 = t16("e16")
            nc.scalar.copy(e16[:], e_r[:])
            i16 = t16("i16")
            nc.scalar.copy(i16[:], i_r[:])

            # T = 2*(G11+G22)
            T2 = t32("T")
            nc.vector.tensor_add(T2[:], e_r[:], i_r[:])
            T16 = t16("T16")
            nc.scalar.copy(T16[:], T2[:])

            # ---- TensorEngine: D-gradient rows --------------------------
            # a,b,c = 2*G00, 2*G01, 2*G02 over the full input chunk
            zc2 = t32("z")
            ab16 = [t16(t) for t in ("a16", "b16", "c16")]
            for c in range(C):
                ps = ps_pool.tile([D, FI], f32, tag="ps", name="ps")
                for k in range(0, FI, 512):
                    n = min(512, FI - k)
                    nc.tensor.matmul(ps[:, k:k + n], dmat[:],
                                     xt[c][:, k:k + n])
                ps_int = v3(ps, h=HIN, w=WP)[:, 1:HIN - 1, 1:WP - 1]
                nc.scalar.copy(v3(ab16[c]), ps_int)
                if c == 0:
                    nc.vector.tensor_add(v3(zc2), ps_int, v3(T2))
            a16, b16, c16 = ab16

            # ---- fp16 product chain (DVE 2x) ----------------------------
            def mul(o, x, y):
                nc.vector.tensor_mul(o[:], x[:], y[:])

            def sub(o, x, y):
                nc.vector.tensor_sub(o[:], x[:], y[:])

            pa, pb = tmp(), tmp()
            mul(pa, e16, i16)
            mul(pb, fd16, hd16)
            N0 = tmp()
            sub(N0, pa, pb)            # 4*(ei - fh)

            pa, pb = tmp(), tmp()
            mul(pa, d16, i16)
            mul(pb, fd16, g16)
            N1 = tmp()
            sub(N1, pa, pb)            # 4*(di - fg)
            Q2 = tmp()
            mul(Q2, b16, N1)           # 8*b(di - fg)

            pa, pb = tmp(), tmp()
            mul(pa, d16, hd16)
            mul(pb, e16, g16)
            N2 = tmp()
            sub(N2, pa, pb)            # 4*(dh - eg)
            Q3 = tmp()
            mul(Q3, c16, N2)           # 8*c(dh - eg)

            Q1 = tmp()
            mul(Q1, a16, N0)           # 8*a(ei - fh)
            Q4 = tmp()
            sub(Q4, Q1, Q2)
            nc.vector.tensor_add(Q4[:], Q4[:], Q3[:])  # R3 = 8*c3, in place
            R3 = Q4

            U1 = tmp()
            mul(U1, a16, T16)          # 4*a(e+i)
            U2 = tmp()
            mul(U2, b16, d16)          # 4*bd
            U3 = tmp()
            mul(U3, c16, g16)          # 4*cg
            nc.vector.tensor_add(U2[:], U2[:], U3[:])  # U4, in place
            U5 = tmp()
            sub(U5, U1, U2)            # 4*c2 - 4*(ei-fh)
            nc.vector.tensor_add(U5[:], U5[:], N0[:])   # V1 = 4*c2, in place
            nc.vector.tensor_scalar_mul(R3[:], R3[:], 0.5)  # 4*c3, in place
            nc.vector.tensor_add(U5[:], U5[:], R3[:])   # V2 = 4*(c2+c3)
            V2 = U5

            # zf = 2*(det-1) = zc2 + 0.5*V2 ; then log(0.5*zf+1), square
            zf2 = t32("zf")
            nc.vector.scalar_tensor_tensor(zf2[:], V2[:], 0.5, zc2[:],
                                           Alu.mult, Alu.add)
            nc.scalar.activation(zf2[:], zf2[:], Act.Ln, bias=1.0, scale=0.5)
            nc.scalar.activation(zf2[:], zf2[:], Act.Square)

            nc.sync.dma_start(
                out_t[:, chi * HCH:(chi + 1) * HCH, :], v3(zf2))

    nc.compile()
    nc.m = get_hw_module(nc.m)
    return nc


def _get_program():
    if "nc" not in _CACHE:
        _CACHE["nc"] = _build_program()
    return _CACHE["nc"]


def make_in_maps(x: np.ndarray):
    x = np.asarray(x, dtype=np.float32)
    # ghost cells: 2*x[edge] - x[edge+1] makes the central difference of the
    # padded array equal the one-sided boundary difference of the original
    xw = np.concatenate(
        [2.0 * x[..., :1] - x[..., 1:2], x,
         2.0 * x[..., -1:] - x[..., -2:-1]], axis=-1)
    xh = np.concatenate(
        [2.0 * xw[:, :, :, :1] - xw[:, :, :, 1:2], xw,
         2.0 * xw[:, :, :, -1:] - xw[:, :, :, -2:-1]], axis=3)
    dmat = _dmat2()
    in_maps = []
    for core in range(NCORES):
        b, hq = divmod(core, HQ)
        slab = np.ascontiguousarray(xh[b, :, :, hq * HL: hq * HL + HL + 2, :])
        in_maps.append({"x": slab, "dmat": dmat})
    return in_maps


def kernel(x: np.ndarray) -> np.ndarray:
    x = np.asarray(x, dtype=np.float32)
    assert x.shape == (B, C, D, H, W)
    in_maps = make_in_maps(x)
    nc = _get_program()
    res = run_bass_kernel_spmd(nc, in_maps, list(range(NCORES)))
    out = np.empty((B, D, H, W), np.float32)
    for core in range(NCORES):
        b, hq = divmod(core, HQ)
        out[b, :, hq * HL:(hq + 1) * HL, :] = res.results[core]["out"]
    return out


if __name__ == "__main__":
    rng = np.random.default_rng(0)
    xt = (rng.standard_normal((B, C, D, H, W)) * 0.05).astype(np.float32)
    y = kernel(xt)
    print("out", y.shape, y.dtype, float(y.mean()))
